# revision 15
# baseline (speedup 1.0000x reference)
"""Trainium2 Bass kernel for GroupedVectorSA (gnn message passing), v2.

Self-contained: accepts FULL inputs (as produced by setup_inputs()), shards
across 8 NeuronCores internally (batch b = core//4, quarter of N = core%4),
runs one SPMD Bass/Tile program via bass_utils.run_bass_kernel_spmd, and
reassembles the full [B, N, C] output.

v2 design (vs v1 baseline at ~701us):
  - k/v are projected ONCE for the full batch (4096 rows) into an SBUF table
    kvT [128, 4096, 4] (channel-partition, point, {k0,k1,v0,v1}); the
    per-neighbor rows (16384 = 16x more) are then fetched with gpsimd
    ap_gather on the otherwise-idle Pool engine, replacing ~8 of 31 matmuls
    per tile plus all per-chunk DMA transposes.
  - feats arrive pre-transposed from the host (featsT), so no
    dma_start_transpose anywhere.
  - One uniform round-robin PSUM pool (8 banks of [128,512] f32) instead of
    single-buffered pools, removing the PE<->Scalar ping-pong stalls.
  - Softmax weights are normalized on the 8-partition e vector (DVE) instead
    of broadcasting reciprocals through PE matmuls; vg + peb is accumulated
    by the PE directly on top of a scalar-engine copy of vg (start=False).
  - Output assembled as bf16, transposed once per 128 points.

Math per core (points n in its quarter, gathers within its full batch):
  q = relu(bn(feats @ wq + bq)); k = relu(bn(feats @ wk + bk)); v = feats @ wv + bv
  pos = coords[n] - coords[idx]   (host-computed, tiny)
  pem = relu(bn(pos @ pm_w1 + pm_b1)) @ pm_w2 + pm_b2
  peb0 = relu(bn(pos @ pb_w1 + pb_b1)) @ pb_w2          (pb_b2 folded)
  rel = (kg - q)*pem + peb0
  w = softmax(relu(bn(rel @ we_w1 + we_b1')) @ we_w2 + we_b2, over S)
  out = sum_s w * (vg + peb0) + pb_b2      (softmax weights sum to 1)

BN is folded on the host into per-channel affine scale/bias (eval mode).
"""

import os
import sys

import numpy as np

try:
    import concourse  # noqa: F401
except ImportError:
    sys.path.insert(0, "/opt/trn_rl_repo")

import ml_dtypes

import concourse.bacc as bacc
import concourse.bass as bass  # noqa: F401
import concourse.mybir as mybir
import concourse.tile as tile
from concourse import bass_utils, library_config

F32 = mybir.dt.float32
BF16 = mybir.dt.bfloat16
FP16 = mybir.dt.float16
I16 = mybir.dt.int16

NP_BF16 = ml_dtypes.bfloat16

EPS = 1e-5
B, N, S, C, G = 2, 4096, 16, 256, 8
NCORES = 8
CPB = NCORES // B          # cores per batch = 4
NLOC = N // CPB            # points per core = 1024
NPT = 32                   # points per compute tile
RT = NPT * S               # rows per compute tile = 512
NTILES = NLOC // NPT       # 32
CHUNK = 1024               # gather chunk (rows)
NCHUNKS = NLOC * S // CHUNK        # 16
TPC = CHUNK // RT                  # tiles per chunk = 2

AO = mybir.AluOpType
AF = mybir.ActivationFunctionType


def _affine(bn_p, lin_b):
    """Fold eval-mode BN (+ preceding linear bias) into scale/bias vectors."""
    bn_p = np.asarray(bn_p, np.float32)
    g, beta, m, v = bn_p[0], bn_p[1], bn_p[2], bn_p[3]
    s = g / np.sqrt(v + EPS)
    t = (np.asarray(lin_b, np.float32) - m) * s + beta
    return s.astype(np.float32), t.astype(np.float32)


def _as_lhst(w):
    """[256, X] -> [128, 2, X] (partition = K within K-tile)."""
    w = np.asarray(w, np.float32)
    return np.ascontiguousarray(w.reshape(2, 128, w.shape[1]).transpose(1, 0, 2))


def _per_part(vec):
    """[256] -> [128, 2]  (channel = j*128 + p)."""
    return np.ascontiguousarray(np.asarray(vec, np.float32).reshape(2, 128).T)


def build_program(ablate=""):
    nc = bacc.Bacc("TRN2", target_bir_lowering=False, debug=False,
                   num_devices=NCORES)

    def din(name, shape, dt):
        return nc.dram_tensor(name, list(shape), dt, kind="ExternalInput")

    featsT_d = din("featsT", [128, 2, N], BF16)
    posT_d = din("posT", [4, NLOC * S], FP16)
    idxw_d = din("idxw", [128, NLOC * S // 16], I16)
    consts = [
        ("wq3", [128, 2, C], BF16), ("wke3", [128, 2, C], BF16),
        ("wv3", [128, 2, C], BF16),
        ("tkT", [128, 2], F32), ("tvT", [128, 2], F32),
        ("sq", [128, 2], F32), ("tq", [128, 2], F32),
        ("w1m", [4, C], BF16), ("w1b", [4, C], BF16),
        ("sh1m", [128, 2], F32), ("th1m", [128, 2], F32),
        ("sh1b", [128, 2], F32), ("th1b", [128, 2], F32),
        ("w2m", [128, 2, C], BF16), ("w2b", [128, 2, C], BF16),
        ("b2m", [128, 2], F32), ("b2bt", [128, 2], F32),
        ("we1", [128, 2, G], BF16), ("fw", [128, 2, G], BF16),
        ("swe", [G, 1], F32), ("twe", [G, 1], F32),
        ("we2", [G, G], BF16), ("web2", [G, 1], F32),
        ("eoh", [G, 2, 128], BF16),
        ("identB", [128, 128], FP16),
    ]
    cdram = {name: din(name, shape, dt) for name, shape, dt in consts}

    out_d = nc.dram_tensor("out", [NLOC, C], FP16, kind="ExternalOutput")

    with tile.TileContext(nc) as tc:
        with (
            tc.tile_pool(name="const", bufs=1) as cpool,
            tc.tile_pool(name="big", bufs=1) as bigpool,
            tc.tile_pool(name="gather", bufs=3) as gpool,
            tc.tile_pool(name="work", bufs=2) as wpool,
            tc.tile_pool(name="h1bp", bufs=3) as hpool,
            tc.tile_pool(name="t2p", bufs=3) as tpool,
            tc.tile_pool(name="valp", bufs=5) as vpool,
            tc.tile_pool(name="small", bufs=4) as spool,
            tc.tile_pool(name="outp", bufs=2) as opool,
            tc.tile_pool(name="ps", bufs=7, space="PSUM") as pspool,
            tc.tile_pool(name="pstr", bufs=1, space="PSUM") as trpool,
        ):
            nc.gpsimd.load_library(library_config.ap_gather)

            csb = {}
            for name, shape, dt in consts:
                t = cpool.tile(list(shape), dt, tag=name)
                nc.sync.dma_start(t[:], cdram[name][:])
                csb[name] = t

            # featsT loaded as 4 slabs so projections can start early
            fT = []
            for pc in range(4):
                t = bigpool.tile([128, 2, N // 4], BF16, tag=f"fT{pc}")
                nc.sync.dma_start(t[:], featsT_d[:, :, pc * (N // 4):(pc + 1) * (N // 4)])
                fT.append(t)
            posT = bigpool.tile([4, NLOC * S], FP16, tag="posT")
            nc.sync.dma_start(posT[:], posT_d[:])
            idxw = bigpool.tile([128, NLOC * S // 16], I16, tag="idxw")
            nc.sync.dma_start(idxw[:], idxw_d[:])

            # ---- phase 1: kvT for the full batch, then qT ------------------
            # kvT[p, n, j]: j in {0,1}: k channels mj=j; j in {2,3}: v, mj=j-2
            kvT = bigpool.tile([128, N, 4], BF16, tag="kvT")
            for ch in range(N // 512):
                slab = fT[ch // 2]
                c0 = (ch % 2) * 512
                for j in range(4):
                    mj = j % 2
                    wname = "wke3" if j < 2 else "wv3"
                    ps = pspool.tile([128, 512], F32, tag="ps")
                    for kt in range(2):
                        nc.tensor.matmul(
                            ps[:],
                            csb[wname][:, kt, mj * 128:(mj + 1) * 128],
                            slab[:, kt, c0:c0 + 512],
                            start=(kt == 0), stop=(kt == 1))
                    dst = kvT[:, ch * 512:(ch + 1) * 512, j]
                    if j < 2:
                        nc.scalar.activation(
                            dst, ps[:], AF.Relu,
                            bias=csb["tkT"][:, mj:mj + 1], scale=1.0)
                    else:
                        nc.vector.tensor_scalar_add(
                            dst, ps[:], csb["tvT"][:, mj:mj + 1])

            # qT[p, n, mj] = q[mj*128+p, qoff+n] (own quarter = columns 0..NLOC)
            qT = bigpool.tile([128, NLOC, 2], BF16, tag="qT")
            for ch in range(NLOC // 512):
                for mj in range(2):
                    ps = pspool.tile([128, 512], F32, tag="ps")
                    for kt in range(2):
                        nc.tensor.matmul(
                            ps[:],
                            csb["wq3"][:, kt, mj * 128:(mj + 1) * 128],
                            fT[ch // 2][:, kt, (ch % 2) * 512:(ch % 2) * 512 + 512],
                            start=(kt == 0), stop=(kt == 1))
                    nc.scalar.activation(
                        qT[:, ch * 512:(ch + 1) * 512, mj], ps[:], AF.Relu,
                        bias=csb["tq"][:, mj:mj + 1], scale=csb["sq"][:, mj:mj + 1])

            # ---- phase 2: software-pipelined gather + attention ------------
            # Stages per tile t (each one slot later than the previous):
            #   F(t):  h1/pem/peb matmuls+acts, t1/t2/val      (slot t)
            #   B1(t): lg matmuls, hw act                      (slot t+1)
            #   B2(t): l2 matmul, e act, esum/recip/en         (slot t+2)
            #   B3(t): eb matmuls, prod, outp, +b2bt [, store] (slot t+3)
            # Per-engine streams then never wait on same-slot cross-engine
            # results, so every engine keeps streaming.
            nt = 0 if ablate == "phase1" else NTILES
            st = {}  # per-tile cross-stage tensors

            def gather(c):
                gkv = gpool.tile([128, CHUNK, 4], BF16, tag="gkv")
                nc.gpsimd.ap_gather(
                    gkv[:], kvT[:],
                    idxw[:, c * (CHUNK // 16):(c + 1) * (CHUNK // 16)],
                    channels=128, num_elems=N, d=4, num_idxs=CHUNK)
                return gkv

            def stage_f(t):
                d = st[t] = {}
                gkv = st[f"g{t // TPC}"]
                rr = slice((t % TPC) * RT, (t % TPC) * RT + RT)
                pt0 = t * NPT
                h1m = wpool.tile([128, 2, RT], BF16, tag="h1m")
                h1b = hpool.tile([128, 2, RT], BF16, tag="h1b")
                for (h1, w1, sh, th) in (
                    (h1m, "w1m", "sh1m", "th1m"),
                    (h1b, "w1b", "sh1b", "th1b"),
                ):
                    for kt2 in range(2):
                        ps = pspool.tile([128, 512], F32, tag="ps")
                        nc.tensor.matmul(
                            ps[:],
                            csb[w1][0:3, kt2 * 128:(kt2 + 1) * 128],
                            posT[0:3, t * RT:(t + 1) * RT],
                            start=True, stop=True)
                        nc.scalar.activation(
                            h1[:, kt2, :], ps[:], AF.Relu,
                            bias=csb[th][:, kt2:kt2 + 1],
                            scale=csb[sh][:, kt2:kt2 + 1])
                pem = []
                for mj in range(2):
                    ps = pspool.tile([128, 512], F32, tag="ps")
                    for kt in range(2):
                        nc.tensor.matmul(
                            ps[:],
                            csb["w2m"][:, kt, mj * 128:(mj + 1) * 128],
                            h1m[:, kt, :],
                            start=(kt == 0), stop=(kt == 1))
                    pem.append(ps)
                t1 = wpool.tile([128, RT, 2], BF16, tag="t1")
                qb = qT[:, pt0:pt0 + NPT, :].unsqueeze(2) \
                    .broadcast_to((128, NPT, S, 2))
                nc.vector.tensor_sub(
                    t1[:].rearrange("p (n s) j -> p n s j", s=S),
                    gkv[:, rr, 0:2].rearrange("p (n s) j -> p n s j", s=S),
                    qb)
                t2 = tpool.tile([128, RT, 2], BF16, tag="t2")
                for mj in range(2):
                    nc.vector.scalar_tensor_tensor(
                        t2[:, :, mj], pem[mj][:],
                        csb["b2m"][:, mj:mj + 1],
                        t1[:, :, mj],
                        op0=AO.add, op1=AO.mult)
                val = vpool.tile([128, 2, RT], BF16, tag="val")
                for mj in range(2):
                    ps = pspool.tile([128, 512], F32, tag="ps")
                    for kt in range(2):
                        nc.tensor.matmul(
                            ps[:],
                            csb["w2b"][:, kt, mj * 128:(mj + 1) * 128],
                            h1b[:, kt, :],
                            start=(kt == 0), stop=(kt == 1))
                    nc.vector.tensor_add(val[:, mj, :], gkv[:, rr, 2 + mj],
                                         ps[:])
                d["h1b"], d["t2"], d["val"] = h1b, t2, val

            def stage_b1(t):
                d = st[t]
                h1b, t2 = d["h1b"], d["t2"]
                lg = pspool.tile([128, 512], F32, tag="ps")
                nc.tensor.matmul(lg[0:G, :], csb["we1"][:, 0, :], t2[:, :, 0],
                                 start=True, stop=False)
                nc.tensor.matmul(lg[0:G, :], csb["we1"][:, 1, :], t2[:, :, 1],
                                 start=False, stop=False)
                nc.tensor.matmul(lg[0:G, :], csb["fw"][:, 0, :], h1b[:, 0, :],
                                 start=False, stop=False)
                nc.tensor.matmul(lg[0:G, :], csb["fw"][:, 1, :], h1b[:, 1, :],
                                 start=False, stop=True)
                hw = spool.tile([G, RT], BF16, tag="hw")
                nc.scalar.activation(hw[:], lg[0:G, :], AF.Relu,
                                     bias=csb["twe"][:], scale=csb["swe"][:])
                d["hw"] = hw

            def stage_b2(t):
                d = st[t]
                l2 = pspool.tile([128, 512], F32, tag="ps")
                nc.tensor.matmul(l2[0:G, :], csb["we2"][:], d["hw"][:],
                                 start=True, stop=True)
                e = spool.tile([G, RT], BF16, tag="e")
                nc.scalar.activation(e[:], l2[0:G, :], AF.Exp,
                                     bias=csb["web2"][:], scale=1.0)
                esum = spool.tile([G, NPT], F32, tag="esum")
                nc.vector.reduce_sum(
                    esum[:], e[:].rearrange("p (n s) -> p n s", s=S),
                    axis=mybir.AxisListType.X)
                rinv = spool.tile([G, NPT], F32, tag="rinv")
                nc.vector.reciprocal(rinv[:], esum[:])
                en = spool.tile([G, RT], FP16, tag="en")
                nc.gpsimd.tensor_mul(
                    en[:].rearrange("p (n s) -> p n s", s=S),
                    e[:].rearrange("p (n s) -> p n s", s=S),
                    rinv[:].unsqueeze(2).broadcast_to((G, NPT, S)))
                d["en"] = en

            def stage_b3(t):
                d = st[t]
                val, en = d["val"], d["en"]
                eb = []
                for mj in range(2):
                    ps = pspool.tile([128, 512], F32, tag="ps")
                    nc.tensor.matmul(ps[:], csb["eoh"][:, mj, :], en[:],
                                     start=True, stop=True)
                    eb.append(ps)
                prod = wpool.tile([128, 2, RT], FP16, tag="prod")
                for mj in range(2):
                    nc.vector.tensor_mul(prod[:, mj, :], val[:, mj, :],
                                         eb[mj][:])
                outp = spool.tile([128, 2, NPT], FP16, tag="outp")
                with nc.allow_low_precision(reason="S=16 sum, fp16 out ok"):
                    nc.vector.reduce_sum(
                        outp[:],
                        prod[:].rearrange("p j (n s) -> p j n s", s=S),
                        axis=mybir.AxisListType.X)
                gi = t // 4
                ti = t % 4
                if ti == 0:
                    st["oacc"] = opool.tile([128, 2, 128], FP16, tag="oacc",
                                            name="oacc")
                oacc = st["oacc"]
                for mj in range(2):
                    nc.vector.tensor_scalar_add(
                        oacc[:, mj, ti * NPT:(ti + 1) * NPT],
                        outp[:, mj, :], csb["b2bt"][:, mj:mj + 1])
                if ti == 3:
                    trp = trpool.tile([128, C], FP16, tag="trp")
                    for mj in range(2):
                        nc.tensor.transpose(
                            trp[:, mj * 128:(mj + 1) * 128],
                            oacc[:, mj, :], csb["identB"][:])
                    orows = opool.tile([128, C], FP16, tag="orows")
                    nc.scalar.copy(orows[:], trp[:])
                    nc.sync.dma_start(
                        out_d[gi * 128:(gi + 1) * 128, :], orows[:])
                del st[t]

            if nt:
                st["g0"] = gather(0)
            for s in range(nt + 3):
                # prefetch next chunk one slot ahead of first use
                cn = s // TPC + 1
                if s < nt and s % TPC == 0 and cn < NCHUNKS:
                    st[f"g{cn}"] = gather(cn)
                if s < nt:
                    stage_f(s)
                if 1 <= s <= nt:
                    stage_b1(s - 1)
                if 2 <= s <= nt + 1:
                    stage_b2(s - 2)
                if 3 <= s <= nt + 2:
                    stage_b3(s - 3)

    nc.compile()
    return nc


def host_prep(inputs):
    """Fold BN, cast/transpose weights, build per-core input maps."""
    f = {k: np.asarray(v) for k, v in inputs.items()}
    feats, coords, index = f["feats"], f["coords"], f["index"]
    index = index.astype(np.int64)

    s_q, t_q = _affine(f["bnq"], f["bq"])
    s_k, t_k = _affine(f["bnk"], f["bk"])
    s_hm, t_hm = _affine(f["pm_bn"], f["pm_b1"])
    s_hb, t_hb = _affine(f["pb_bn"], f["pb_b1"])

    b2b_we = np.asarray(f["pb_b2"], np.float32) @ np.asarray(f["we_w1"], np.float32)
    s_we, t_we = _affine(f["we_bn"], np.asarray(f["we_b1"], np.float32) + b2b_we)

    wk_eff = np.asarray(f["wk"], np.float32) * s_k[None, :]
    F_mat = np.asarray(f["pb_w2"], np.float32) @ np.asarray(f["we_w1"], np.float32)

    eoh = np.zeros((G, 2, 128), np.float32)
    for g in range(G):
        j, p0 = divmod(g * 32, 128)
        eoh[g, j, p0:p0 + 32] = 1.0

    shared = {
        "wq3": _as_lhst(f["wq"]).astype(NP_BF16),
        "wke3": _as_lhst(wk_eff).astype(NP_BF16),
        "wv3": _as_lhst(f["wv"]).astype(NP_BF16),
        "tkT": _per_part(t_k),
        "tvT": _per_part(np.asarray(f["bv"], np.float32)),
        "sq": _per_part(s_q), "tq": _per_part(t_q),
        "w1m": np.concatenate([np.asarray(f["pm_w1"], np.float32),
                               np.zeros((1, C), np.float32)], 0).astype(NP_BF16),
        "w1b": np.concatenate([np.asarray(f["pb_w1"], np.float32),
                               np.zeros((1, C), np.float32)], 0).astype(NP_BF16),
        "sh1m": _per_part(s_hm), "th1m": _per_part(t_hm),
        "sh1b": _per_part(s_hb), "th1b": _per_part(t_hb),
        "w2m": _as_lhst(f["pm_w2"]).astype(NP_BF16),
        "w2b": _as_lhst(f["pb_w2"]).astype(NP_BF16),
        "b2m": _per_part(f["pm_b2"]),
        "b2bt": _per_part(f["pb_b2"]),
        "we1": _as_lhst(f["we_w1"]).astype(NP_BF16),
        "fw": _as_lhst(F_mat).astype(NP_BF16),
        "swe": s_we.reshape(G, 1), "twe": t_we.reshape(G, 1),
        "we2": np.asarray(f["we_w2"], np.float32).astype(NP_BF16),
        "web2": np.asarray(f["we_b2"], np.float32).reshape(G, 1),
        "eoh": eoh.astype(NP_BF16),
        "identB": np.eye(128, dtype=np.float16),
    }

    in_maps = []
    for core in range(NCORES):
        b, qc = divmod(core, CPB)
        qoff = qc * NLOC
        # Rotate the batch's points so this core's quarter sits at columns
        # 0..NLOC (the SPMD program projects qT from columns 0..NLOC), and
        # remap gather indices through the same permutation.
        order = np.concatenate([
            np.arange(qoff, qoff + NLOC),
            np.arange(0, qoff),
            np.arange(qoff + NLOC, N)])
        perm = np.empty(N, np.int64)
        perm[order] = np.arange(N)
        fb32 = np.asarray(feats[b], np.float32)[order]
        # featsT[p, kt, n] = feats[order[n], kt*128+p]
        featsT = np.ascontiguousarray(
            fb32.T.reshape(2, 128, N).transpose(1, 0, 2)).astype(NP_BF16)
        idx = perm[index[b, qoff:qoff + NLOC, :].reshape(-1)]
        cb = np.asarray(coords[b], np.float32)[order]
        pos = cb[:NLOC][:, None, :] - cb[idx.reshape(NLOC, S)]
        posT = np.zeros((4, NLOC * S), np.float16)
        posT[:3, :] = pos.reshape(NLOC * S, 3).T
        # wrapped indices: idxw[p, fo] = idx[fo*16 + p%16], replicated x8
        idxw = np.tile(
            np.ascontiguousarray(idx.reshape(-1, 16).T.astype(np.int16)),
            (8, 1))
        m = dict(shared)
        m["featsT"] = featsT
        m["posT"] = posT
        m["idxw"] = idxw
        in_maps.append(m)
    return in_maps


_NC_CACHE = {}


def _get_program():
    ablate = os.environ.get("KERNEL_ABLATE", "")
    key = "nc" + ablate
    if key not in _NC_CACHE:
        _NC_CACHE[key] = build_program(ablate)
    return _NC_CACHE[key]


def kernel(**inputs):
    nc = _get_program()
    in_maps = host_prep(inputs)
    res = bass_utils.run_bass_kernel_spmd(
        nc, in_maps, list(range(NCORES)),
        trace=bool(int(os.environ.get("KERNEL_TRACE", "0"))))
    _NC_CACHE["last_results"] = res
    out = np.zeros((B, N, C), np.float32)
    for core in range(NCORES):
        b, qc = divmod(core, CPB)
        out[b, qc * NLOC:(qc + 1) * NLOC, :] = res.results[core]["out"]
    return out


# revision 16
# speedup vs baseline: 1.4205x; 1.4205x over previous
"""Trainium2 Bass kernel for GroupedVectorSA (gnn message passing), v2.

Self-contained: accepts FULL inputs (as produced by setup_inputs()), shards
across 8 NeuronCores internally (batch b = core//4, quarter of N = core%4),
runs one SPMD Bass/Tile program via bass_utils.run_bass_kernel_spmd, and
reassembles the full [B, N, C] output.

v2 design (vs v1 baseline at ~701us):
  - k/v are projected ONCE for the full batch (4096 rows) into an SBUF table
    kvT [128, 4096, 4] (channel-partition, point, {k0,k1,v0,v1}); the
    per-neighbor rows (16384 = 16x more) are then fetched with gpsimd
    ap_gather on the otherwise-idle Pool engine, replacing ~8 of 31 matmuls
    per tile plus all per-chunk DMA transposes.
  - feats arrive pre-transposed from the host (featsT), so no
    dma_start_transpose anywhere.
  - One uniform round-robin PSUM pool (8 banks of [128,512] f32) instead of
    single-buffered pools, removing the PE<->Scalar ping-pong stalls.
  - Softmax weights are normalized on the 8-partition e vector (DVE) instead
    of broadcasting reciprocals through PE matmuls; vg + peb is accumulated
    by the PE directly on top of a scalar-engine copy of vg (start=False).
  - Output assembled as bf16, transposed once per 128 points.

Math per core (points n in its quarter, gathers within its full batch):
  q = relu(bn(feats @ wq + bq)); k = relu(bn(feats @ wk + bk)); v = feats @ wv + bv
  pos = coords[n] - coords[idx]   (host-computed, tiny)
  pem = relu(bn(pos @ pm_w1 + pm_b1)) @ pm_w2 + pm_b2
  peb0 = relu(bn(pos @ pb_w1 + pb_b1)) @ pb_w2          (pb_b2 folded)
  rel = (kg - q)*pem + peb0
  w = softmax(relu(bn(rel @ we_w1 + we_b1')) @ we_w2 + we_b2, over S)
  out = sum_s w * (vg + peb0) + pb_b2      (softmax weights sum to 1)

BN is folded on the host into per-channel affine scale/bias (eval mode).
"""

import os
import sys

import numpy as np

try:
    import concourse  # noqa: F401
except ImportError:
    sys.path.insert(0, "/opt/trn_rl_repo")

import ml_dtypes

import concourse.bacc as bacc
import concourse.bass as bass  # noqa: F401
import concourse.mybir as mybir
import concourse.tile as tile
from concourse import bass_utils, library_config

F32 = mybir.dt.float32
BF16 = mybir.dt.bfloat16
FP16 = mybir.dt.float16
I16 = mybir.dt.int16

NP_BF16 = ml_dtypes.bfloat16

EPS = 1e-5
B, N, S, C, G = 2, 4096, 16, 256, 8
NCORES = 8
CPB = NCORES // B          # cores per batch = 4
NLOC = N // CPB            # points per core = 1024
NPT = 32                   # points per compute tile
RT = NPT * S               # rows per compute tile = 512
NTILES = NLOC // NPT       # 32
CHUNK = 1024               # gather chunk (rows)
NCHUNKS = NLOC * S // CHUNK        # 16
TPC = CHUNK // RT                  # tiles per chunk = 2

AO = mybir.AluOpType
AF = mybir.ActivationFunctionType


def _affine(bn_p, lin_b):
    """Fold eval-mode BN (+ preceding linear bias) into scale/bias vectors."""
    bn_p = np.asarray(bn_p, np.float32)
    g, beta, m, v = bn_p[0], bn_p[1], bn_p[2], bn_p[3]
    s = g / np.sqrt(v + EPS)
    t = (np.asarray(lin_b, np.float32) - m) * s + beta
    return s.astype(np.float32), t.astype(np.float32)


def _as_lhst(w):
    """[256, X] -> [128, 2, X] (partition = K within K-tile)."""
    w = np.asarray(w, np.float32)
    return np.ascontiguousarray(w.reshape(2, 128, w.shape[1]).transpose(1, 0, 2))


def _per_part(vec):
    """[256] -> [128, 2]  (channel = j*128 + p)."""
    return np.ascontiguousarray(np.asarray(vec, np.float32).reshape(2, 128).T)


def build_program(ablate=""):
    nc = bacc.Bacc("TRN2", target_bir_lowering=False, debug=False,
                   num_devices=NCORES)

    def din(name, shape, dt):
        return nc.dram_tensor(name, list(shape), dt, kind="ExternalInput")

    featsT_d = din("featsT", [128, 2, N], BF16)
    posT_d = din("posT", [4, NLOC * S], FP16)
    idxw_d = din("idxw", [128, NLOC * S // 16], I16)
    consts = [
        ("wq3", [128, 2, C], BF16), ("wke3", [128, 2, C], BF16),
        ("wv3", [128, 2, C], BF16),
        ("tkT", [128, 2], F32), ("tvT", [128, 2], F32),
        ("sq", [128, 2], F32), ("tq", [128, 2], F32),
        ("w1m", [4, C], BF16), ("w1b", [4, C], BF16),
        ("sh1m", [128, 2], F32), ("th1m", [128, 2], F32),
        ("sh1b", [128, 2], F32), ("th1b", [128, 2], F32),
        ("w2m", [128, 2, C], BF16), ("w2b", [128, 2, C], BF16),
        ("b2m", [128, 2], F32), ("b2bt", [128, 2], F32),
        ("we1", [128, 2, G], BF16), ("fw", [128, 2, G], BF16),
        ("swe", [G, 1], F32), ("twe", [G, 1], F32),
        ("we2", [G, G], BF16), ("web2", [G, 1], F32),
        ("eoh", [G, 2, 128], BF16),
        ("identB", [128, 128], FP16),
    ]
    cdram = {name: din(name, shape, dt) for name, shape, dt in consts}

    out_d = nc.dram_tensor("out", [NLOC, C], FP16, kind="ExternalOutput")

    with tile.TileContext(nc) as tc:
        with (
            tc.tile_pool(name="const", bufs=1) as cpool,
            tc.tile_pool(name="big", bufs=1) as bigpool,
            tc.tile_pool(name="gather", bufs=3) as gpool,
            tc.tile_pool(name="work", bufs=2) as wpool,
            tc.tile_pool(name="h1bp", bufs=3) as hpool,
            tc.tile_pool(name="t2p", bufs=3) as tpool,
            tc.tile_pool(name="valp", bufs=5) as vpool,
            tc.tile_pool(name="small", bufs=4) as spool,
            tc.tile_pool(name="outp", bufs=2) as opool,
            tc.tile_pool(name="ps", bufs=7, space="PSUM") as pspool,
            tc.tile_pool(name="pstr", bufs=1, space="PSUM") as trpool,
        ):
            nc.gpsimd.load_library(library_config.ap_gather)

            csb = {}
            for name, shape, dt in consts:
                t = cpool.tile(list(shape), dt, tag=name)
                nc.sync.dma_start(t[:], cdram[name][:])
                csb[name] = t

            # featsT loaded as 4 slabs so projections can start early
            fT = []
            for pc in range(4):
                t = bigpool.tile([128, 2, N // 4], BF16, tag=f"fT{pc}")
                nc.sync.dma_start(t[:], featsT_d[:, :, pc * (N // 4):(pc + 1) * (N // 4)])
                fT.append(t)
            posT = bigpool.tile([4, NLOC * S], FP16, tag="posT")
            nc.sync.dma_start(posT[:], posT_d[:])
            idxw = bigpool.tile([128, NLOC * S // 16], I16, tag="idxw")
            nc.sync.dma_start(idxw[:], idxw_d[:])

            # ---- phase 1: kvT for the full batch, then qT ------------------
            # kvT[p, n, j]: j in {0,1}: k channels mj=j; j in {2,3}: v, mj=j-2
            kvT = bigpool.tile([128, N, 4], BF16, tag="kvT")
            for ch in range(N // 512):
                slab = fT[ch // 2]
                c0 = (ch % 2) * 512
                for j in range(4):
                    mj = j % 2
                    wname = "wke3" if j < 2 else "wv3"
                    ps = pspool.tile([128, 512], F32, tag="ps")
                    for kt in range(2):
                        nc.tensor.matmul(
                            ps[:],
                            csb[wname][:, kt, mj * 128:(mj + 1) * 128],
                            slab[:, kt, c0:c0 + 512],
                            start=(kt == 0), stop=(kt == 1))
                    dst = kvT[:, ch * 512:(ch + 1) * 512, j]
                    if j < 2:
                        nc.scalar.activation(
                            dst, ps[:], AF.Relu,
                            bias=csb["tkT"][:, mj:mj + 1], scale=1.0)
                    else:
                        nc.vector.tensor_scalar_add(
                            dst, ps[:], csb["tvT"][:, mj:mj + 1])

            # qT[p, n, mj] = q[mj*128+p, qoff+n] (own quarter = columns 0..NLOC)
            qT = bigpool.tile([128, NLOC, 2], BF16, tag="qT")
            for ch in range(NLOC // 512):
                for mj in range(2):
                    ps = pspool.tile([128, 512], F32, tag="ps")
                    for kt in range(2):
                        nc.tensor.matmul(
                            ps[:],
                            csb["wq3"][:, kt, mj * 128:(mj + 1) * 128],
                            fT[ch // 2][:, kt, (ch % 2) * 512:(ch % 2) * 512 + 512],
                            start=(kt == 0), stop=(kt == 1))
                    nc.scalar.activation(
                        qT[:, ch * 512:(ch + 1) * 512, mj], ps[:], AF.Relu,
                        bias=csb["tq"][:, mj:mj + 1], scale=csb["sq"][:, mj:mj + 1])

            # ---- phase 2: software-pipelined gather + attention ------------
            # Stages per tile t (each one slot later than the previous):
            #   F(t):  h1/pem/peb matmuls+acts, t1/t2/val      (slot t)
            #   B1(t): lg matmuls, hw act                      (slot t+1)
            #   B2(t): l2 matmul, e act, esum/recip/en         (slot t+2)
            #   B3(t): eb matmuls, prod, outp, +b2bt [, store] (slot t+3)
            # Per-engine streams then never wait on same-slot cross-engine
            # results, so every engine keeps streaming.
            nt = 0 if ablate == "phase1" else NTILES
            st = {}  # per-tile cross-stage tensors

            def gather(c):
                gkv = gpool.tile([128, CHUNK, 4], BF16, tag="gkv")
                nc.gpsimd.ap_gather(
                    gkv[:], kvT[:],
                    idxw[:, c * (CHUNK // 16):(c + 1) * (CHUNK // 16)],
                    channels=128, num_elems=N, d=4, num_idxs=CHUNK)
                return gkv

            def stage_f(t):
                d = st[t] = {}
                gkv = st[f"g{t // TPC}"]
                rr = slice((t % TPC) * RT, (t % TPC) * RT + RT)
                pt0 = t * NPT
                h1m = wpool.tile([128, 2, RT], BF16, tag="h1m")
                h1b = hpool.tile([128, 2, RT], BF16, tag="h1b")
                for (h1, w1, sh, th) in (
                    (h1m, "w1m", "sh1m", "th1m"),
                    (h1b, "w1b", "sh1b", "th1b"),
                ):
                    for kt2 in range(2):
                        ps = pspool.tile([128, 512], F32, tag="ps")
                        nc.tensor.matmul(
                            ps[:],
                            csb[w1][0:3, kt2 * 128:(kt2 + 1) * 128],
                            posT[0:3, t * RT:(t + 1) * RT],
                            start=True, stop=True)
                        nc.scalar.activation(
                            h1[:, kt2, :], ps[:], AF.Relu,
                            bias=csb[th][:, kt2:kt2 + 1],
                            scale=csb[sh][:, kt2:kt2 + 1])
                pem = []
                for mj in range(2):
                    ps = pspool.tile([128, 512], F32, tag="ps")
                    for kt in range(2):
                        nc.tensor.matmul(
                            ps[:],
                            csb["w2m"][:, kt, mj * 128:(mj + 1) * 128],
                            h1m[:, kt, :],
                            start=(kt == 0), stop=(kt == 1))
                    pem.append(ps)
                t1 = wpool.tile([128, RT, 2], BF16, tag="t1")
                qb = qT[:, pt0:pt0 + NPT, :].unsqueeze(2) \
                    .broadcast_to((128, NPT, S, 2))
                nc.vector.tensor_sub(
                    t1[:].rearrange("p (n s) j -> p n s j", s=S),
                    gkv[:, rr, 0:2].rearrange("p (n s) j -> p n s j", s=S),
                    qb)
                t2 = tpool.tile([128, RT, 2], BF16, tag="t2")
                for mj in range(2):
                    nc.vector.scalar_tensor_tensor(
                        t2[:, :, mj], pem[mj][:],
                        csb["b2m"][:, mj:mj + 1],
                        t1[:, :, mj],
                        op0=AO.add, op1=AO.mult)
                val = vpool.tile([128, 2, RT], BF16, tag="val")
                for mj in range(2):
                    ps = pspool.tile([128, 512], F32, tag="ps")
                    for kt in range(2):
                        nc.tensor.matmul(
                            ps[:],
                            csb["w2b"][:, kt, mj * 128:(mj + 1) * 128],
                            h1b[:, kt, :],
                            start=(kt == 0), stop=(kt == 1))
                    nc.vector.tensor_add(val[:, mj, :], gkv[:, rr, 2 + mj],
                                         ps[:])
                d["h1b"], d["t2"], d["val"] = h1b, t2, val

            def stage_b1(t):
                d = st[t]
                h1b, t2 = d["h1b"], d["t2"]
                lg = pspool.tile([128, 512], F32, tag="ps")
                nc.tensor.matmul(lg[0:G, :], csb["we1"][:, 0, :], t2[:, :, 0],
                                 start=True, stop=False)
                nc.tensor.matmul(lg[0:G, :], csb["we1"][:, 1, :], t2[:, :, 1],
                                 start=False, stop=False)
                nc.tensor.matmul(lg[0:G, :], csb["fw"][:, 0, :], h1b[:, 0, :],
                                 start=False, stop=False)
                nc.tensor.matmul(lg[0:G, :], csb["fw"][:, 1, :], h1b[:, 1, :],
                                 start=False, stop=True)
                hw = spool.tile([G, RT], BF16, tag="hw")
                nc.scalar.activation(hw[:], lg[0:G, :], AF.Relu,
                                     bias=csb["twe"][:], scale=csb["swe"][:])
                d["hw"] = hw

            def stage_b2(t):
                d = st[t]
                l2 = pspool.tile([128, 512], F32, tag="ps")
                nc.tensor.matmul(l2[0:G, :], csb["we2"][:], d["hw"][:],
                                 start=True, stop=True)
                e = spool.tile([G, RT], BF16, tag="e")
                nc.scalar.activation(e[:], l2[0:G, :], AF.Exp,
                                     bias=csb["web2"][:], scale=1.0)
                esum = spool.tile([G, NPT], F32, tag="esum")
                nc.vector.reduce_sum(
                    esum[:], e[:].rearrange("p (n s) -> p n s", s=S),
                    axis=mybir.AxisListType.X)
                rinv = spool.tile([G, NPT], F32, tag="rinv")
                nc.vector.reciprocal(rinv[:], esum[:])
                en = spool.tile([G, RT], FP16, tag="en")
                nc.vector.tensor_mul(
                    en[:].rearrange("p (n s) -> p n s", s=S),
                    e[:].rearrange("p (n s) -> p n s", s=S),
                    rinv[:].unsqueeze(2).broadcast_to((G, NPT, S)))
                d["en"] = en

            def stage_b3(t):
                d = st[t]
                val, en = d["val"], d["en"]
                eb = []
                for mj in range(2):
                    ps = pspool.tile([128, 512], F32, tag="ps")
                    nc.tensor.matmul(ps[:], csb["eoh"][:, mj, :], en[:],
                                     start=True, stop=True)
                    eb.append(ps)
                prod = wpool.tile([128, 2, RT], FP16, tag="prod")
                for mj in range(2):
                    nc.vector.tensor_mul(prod[:, mj, :], val[:, mj, :],
                                         eb[mj][:])
                outp = spool.tile([128, 2, NPT], FP16, tag="outp")
                with nc.allow_low_precision(reason="S=16 sum, fp16 out ok"):
                    nc.vector.reduce_sum(
                        outp[:],
                        prod[:].rearrange("p j (n s) -> p j n s", s=S),
                        axis=mybir.AxisListType.X)
                gi = t // 4
                ti = t % 4
                if ti == 0:
                    st["oacc"] = opool.tile([128, 2, 128], FP16, tag="oacc",
                                            name="oacc")
                oacc = st["oacc"]
                for mj in range(2):
                    nc.vector.tensor_scalar_add(
                        oacc[:, mj, ti * NPT:(ti + 1) * NPT],
                        outp[:, mj, :], csb["b2bt"][:, mj:mj + 1])
                if ti == 3:
                    trp = trpool.tile([128, C], FP16, tag="trp")
                    for mj in range(2):
                        nc.tensor.transpose(
                            trp[:, mj * 128:(mj + 1) * 128],
                            oacc[:, mj, :], csb["identB"][:])
                    orows = opool.tile([128, C], FP16, tag="orows")
                    nc.scalar.copy(orows[:], trp[:])
                    nc.sync.dma_start(
                        out_d[gi * 128:(gi + 1) * 128, :], orows[:])
                del st[t]

            if nt:
                st["g0"] = gather(0)
            for s in range(nt + 3):
                # prefetch next chunk one slot ahead of first use
                cn = s // TPC + 1
                if s < nt and s % TPC == 0 and cn < NCHUNKS:
                    st[f"g{cn}"] = gather(cn)
                if s < nt:
                    stage_f(s)
                if 1 <= s <= nt:
                    stage_b1(s - 1)
                if 2 <= s <= nt + 1:
                    stage_b2(s - 2)
                if 3 <= s <= nt + 2:
                    stage_b3(s - 3)

    nc.compile()
    return nc


def host_prep(inputs):
    """Fold BN, cast/transpose weights, build per-core input maps."""
    f = {k: np.asarray(v) for k, v in inputs.items()}
    feats, coords, index = f["feats"], f["coords"], f["index"]
    index = index.astype(np.int64)

    s_q, t_q = _affine(f["bnq"], f["bq"])
    s_k, t_k = _affine(f["bnk"], f["bk"])
    s_hm, t_hm = _affine(f["pm_bn"], f["pm_b1"])
    s_hb, t_hb = _affine(f["pb_bn"], f["pb_b1"])

    b2b_we = np.asarray(f["pb_b2"], np.float32) @ np.asarray(f["we_w1"], np.float32)
    s_we, t_we = _affine(f["we_bn"], np.asarray(f["we_b1"], np.float32) + b2b_we)

    wk_eff = np.asarray(f["wk"], np.float32) * s_k[None, :]
    F_mat = np.asarray(f["pb_w2"], np.float32) @ np.asarray(f["we_w1"], np.float32)

    eoh = np.zeros((G, 2, 128), np.float32)
    for g in range(G):
        j, p0 = divmod(g * 32, 128)
        eoh[g, j, p0:p0 + 32] = 1.0

    shared = {
        "wq3": _as_lhst(f["wq"]).astype(NP_BF16),
        "wke3": _as_lhst(wk_eff).astype(NP_BF16),
        "wv3": _as_lhst(f["wv"]).astype(NP_BF16),
        "tkT": _per_part(t_k),
        "tvT": _per_part(np.asarray(f["bv"], np.float32)),
        "sq": _per_part(s_q), "tq": _per_part(t_q),
        "w1m": np.concatenate([np.asarray(f["pm_w1"], np.float32),
                               np.zeros((1, C), np.float32)], 0).astype(NP_BF16),
        "w1b": np.concatenate([np.asarray(f["pb_w1"], np.float32),
                               np.zeros((1, C), np.float32)], 0).astype(NP_BF16),
        "sh1m": _per_part(s_hm), "th1m": _per_part(t_hm),
        "sh1b": _per_part(s_hb), "th1b": _per_part(t_hb),
        "w2m": _as_lhst(f["pm_w2"]).astype(NP_BF16),
        "w2b": _as_lhst(f["pb_w2"]).astype(NP_BF16),
        "b2m": _per_part(f["pm_b2"]),
        "b2bt": _per_part(f["pb_b2"]),
        "we1": _as_lhst(f["we_w1"]).astype(NP_BF16),
        "fw": _as_lhst(F_mat).astype(NP_BF16),
        "swe": s_we.reshape(G, 1), "twe": t_we.reshape(G, 1),
        "we2": np.asarray(f["we_w2"], np.float32).astype(NP_BF16),
        "web2": np.asarray(f["we_b2"], np.float32).reshape(G, 1),
        "eoh": eoh.astype(NP_BF16),
        "identB": np.eye(128, dtype=np.float16),
    }

    in_maps = []
    for core in range(NCORES):
        b, qc = divmod(core, CPB)
        qoff = qc * NLOC
        # Rotate the batch's points so this core's quarter sits at columns
        # 0..NLOC (the SPMD program projects qT from columns 0..NLOC), and
        # remap gather indices through the same permutation.
        order = np.concatenate([
            np.arange(qoff, qoff + NLOC),
            np.arange(0, qoff),
            np.arange(qoff + NLOC, N)])
        perm = np.empty(N, np.int64)
        perm[order] = np.arange(N)
        fb32 = np.asarray(feats[b], np.float32)[order]
        # featsT[p, kt, n] = feats[order[n], kt*128+p]
        featsT = np.ascontiguousarray(
            fb32.T.reshape(2, 128, N).transpose(1, 0, 2)).astype(NP_BF16)
        idx = perm[index[b, qoff:qoff + NLOC, :].reshape(-1)]
        cb = np.asarray(coords[b], np.float32)[order]
        pos = cb[:NLOC][:, None, :] - cb[idx.reshape(NLOC, S)]
        posT = np.zeros((4, NLOC * S), np.float16)
        posT[:3, :] = pos.reshape(NLOC * S, 3).T
        # wrapped indices: idxw[p, fo] = idx[fo*16 + p%16], replicated x8
        idxw = np.tile(
            np.ascontiguousarray(idx.reshape(-1, 16).T.astype(np.int16)),
            (8, 1))
        m = dict(shared)
        m["featsT"] = featsT
        m["posT"] = posT
        m["idxw"] = idxw
        in_maps.append(m)
    return in_maps


_NC_CACHE = {}


def _get_program():
    ablate = os.environ.get("KERNEL_ABLATE", "")
    key = "nc" + ablate
    if key not in _NC_CACHE:
        _NC_CACHE[key] = build_program(ablate)
    return _NC_CACHE[key]


def kernel(**inputs):
    nc = _get_program()
    in_maps = host_prep(inputs)
    res = bass_utils.run_bass_kernel_spmd(
        nc, in_maps, list(range(NCORES)),
        trace=bool(int(os.environ.get("KERNEL_TRACE", "0"))))
    _NC_CACHE["last_results"] = res
    out = np.zeros((B, N, C), np.float32)
    for core in range(NCORES):
        b, qc = divmod(core, CPB)
        out[b, qc * NLOC:(qc + 1) * NLOC, :] = res.results[core]["out"]
    return out
